# revision 15
# baseline (speedup 1.0000x reference)
"""Fused AttnBlock kernel for 8 Trainium2 NeuronCores.

Problem: q = LN_head(x1 @ wq + bq), k = LN_head(x2 @ wk + bk), v = x2 @ wv + bv,
out = softmax(q k^T / sqrt(D)) v, with B=4, N=2048, C=1024, H=16, D=64.

Sharding: data-parallel over batch (4) x tensor-parallel over head groups (2).
Each core handles one (batch, head-group) pair fully locally: its 8 heads'
columns of wq/wkv are contiguous slices, so there are no collectives; the host
scatters inputs and gathers/transposes outputs.

Per-core dataflow (all matmuls in float32r = full PE rate at N>=256):
  - host pre-transposes x1/x2 so the contraction dim (C) lands on partitions
  - projection: q/k/v [n,512] tiles via PSUM accumulation over 8 K-tiles
  - per-head LayerNorm on DVE (grouped bn_stats), then PE-transpose q,k to
    [d, n] layout for attention
  - scores^T[m,n] = k_h^T q_h per m-tile; ACT computes exp(s/8) PSUM->SBUF
    (LN bounds |s| <= 8 so the max-subtraction is unnecessary)
  - v is augmented with a ones column, so the PV matmul's row 64 accumulates
    the softmax denominators for free; normalize = reciprocal + broadcast-DMA
    + one multiply; DMA out in [d, n] layout, host transposes back
"""

import os
import sys

for _p in ("/opt/trn_rl_repo",):
    if _p not in sys.path:
        sys.path.insert(0, _p)

import numpy as np

import concourse.bass as bass
import concourse.mybir as mybir
import concourse.tile as tile
from concourse.bass_utils import run_bass_kernel_spmd

F32 = mybir.dt.float32
F32R = mybir.dt.float32r

B = 4
NSEQ = 2048
DIM = 1024
NHEADS = 16
HDIM = 64
EPS = 1e-5

NCORES = 8
HG = 8            # heads per core
JW = HG * HDIM    # 512 output channels per core
KT = DIM // 128   # 8 contraction tiles for the projections


def split_multi_waits(nc, maxw=1):
    # TRN2 instructions carry a single sem-wait slot; this walrus build rejects
    # more. Tile's exit drain accumulates one wait per engine/DMA queue, so
    # hoist the excess onto injected NoOps just before the offending inst.
    for bb in nc.main_func.blocks:
        new_insts = []
        for inst in bb.instructions:
            si = inst.sync_info
            if si is not None and si.on_wait and len(si.on_wait) > maxw:
                waits = list(si.on_wait)
                extra, keep = waits[:-maxw], waits[-maxw:]
                for ci in range(0, len(extra), maxw):
                    nop = mybir.InstNoOp(
                        name=nc.get_next_instruction_name(), ins=[], outs=[],
                        sync_info=mybir.SyncInfo(
                            on_wait=extra[ci:ci + maxw], on_update=[]),
                    )
                    nop.engine = inst.engine
                    new_insts.append(nop)
                    nc.register_instruction(nop, overwrite=True)
                inst.sync_info = mybir.SyncInfo(
                    on_wait=keep, on_update=list(si.on_update))
            new_insts.append(inst)
        bb.instructions[:] = new_insts


def build(n_seq=NSEQ, has_bq=False, has_bkv=False, has_gbq=False, has_gbk=False):
    nt_n = n_seq // 128        # n tiles (16)
    sw = min(1024, n_seq)      # s-tile width (ACT exp granularity)
    nblk = n_seq // sw         # n blocks per head
    nch = sw // 512            # 512-wide output chunks per block
    scale = 1.0 / np.sqrt(HDIM)

    nc = bass.Bass()
    x1t = nc.dram_tensor("x1t", [DIM, n_seq], F32, kind="ExternalInput")
    x2t = nc.dram_tensor("x2t", [DIM, n_seq], F32, kind="ExternalInput")
    wq_d = nc.dram_tensor("wq", [DIM, JW], F32, kind="ExternalInput")
    wk_d = nc.dram_tensor("wk", [DIM, JW], F32, kind="ExternalInput")
    wv_d = nc.dram_tensor("wv", [DIM, JW], F32, kind="ExternalInput")
    eye_d = nc.dram_tensor("eye", [128, 128], F32, kind="ExternalInput")
    onesv_d = nc.dram_tensor("onesv", [128], F32, kind="ExternalInput")
    if has_bq:
        bq_d = nc.dram_tensor("bq", [JW], F32, kind="ExternalInput")
    if has_bkv:
        bk_d = nc.dram_tensor("bk", [JW], F32, kind="ExternalInput")
        bv_d = nc.dram_tensor("bv", [JW], F32, kind="ExternalInput")
    if has_gbq:
        gq_d = nc.dram_tensor("gq", [JW], F32, kind="ExternalInput")
        betq_d = nc.dram_tensor("betq", [JW], F32, kind="ExternalInput")
    if has_gbk:
        gk_d = nc.dram_tensor("gk", [JW], F32, kind="ExternalInput")
        betk_d = nc.dram_tensor("betk", [JW], F32, kind="ExternalInput")
    out_d = nc.dram_tensor("outT", [JW, n_seq], F32, kind="ExternalOutput")

    def bcast_from_dram(pool, vec_d, name):
        t = pool.tile([128, JW], F32, name=name)
        src = bass.AP(tensor=vec_d.tensor, offset=vec_d.offset,
                      ap=[[0, 128]] + list(vec_d.ap))
        nc.sync.dma_start(out=t, in_=src)
        return t

    with tile.TileContext(nc) as tc:
        with tc.tile_pool(name="persist", bufs=1) as persist:
            qT = persist.tile([128, 4, n_seq], F32R)   # [j, n] post-LN q
            kT = persist.tile([128, 4, n_seq], F32R)
            vA = persist.tile([128, nt_n, HG, HDIM + 1], F32R)  # v + ones col
            eye_sb = persist.tile([128, 128], F32)
            eps_sb = persist.tile([128, 1], F32)
            ones64 = persist.tile([1, 64], F32R)
            nc.sync.dma_start(out=eye_sb, in_=eye_d[:, :])
            nc.vector.memset(eps_sb, EPS)
            # memset can't target f32r tiles on this compiler; DMA ones instead
            ones64_src = bass.AP(tensor=onesv_d, offset=0,
                                 ap=[[0, 1], [1, 64]])
            nc.sync.dma_start(out=ones64, in_=ones64_src.bitcast(F32R))
            vones_src = bass.AP(tensor=onesv_d, offset=0,
                                ap=[[0, 128], [0, nt_n * HG], [1, 1]])
            nc.sync.dma_start(out=vA[:, :, :, HDIM:HDIM + 1],
                              in_=vones_src.bitcast(F32R))

            bqb = bcast_from_dram(persist, bq_d[:], "bqb") if has_bq else None
            bkb = bcast_from_dram(persist, bk_d[:], "bkb") if has_bkv else None
            bvb = bcast_from_dram(persist, bv_d[:], "bvb") if has_bkv else None
            gqb = bcast_from_dram(persist, gq_d[:], "gqb") if has_gbq else None
            btqb = bcast_from_dram(persist, betq_d[:], "btqb") if has_gbq else None
            gkb = bcast_from_dram(persist, gk_d[:], "gkb") if has_gbk else None
            btkb = bcast_from_dram(persist, betk_d[:], "btkb") if has_gbk else None

            # ---------------- projection + LN + transpose ----------------
            with tc.tile_pool(name="wpool", bufs=1) as wpool, \
                 tc.tile_pool(name="xc", bufs=3) as xc_pool, \
                 tc.tile_pool(name="lnb", bufs=3) as ln_pool, \
                 tc.tile_pool(name="stats", bufs=4) as st_pool, \
                 tc.tile_pool(name="pps", bufs=4, space="PSUM") as proj_ps, \
                 tc.tile_pool(name="tps", bufs=4, space="PSUM") as tp_ps:

                w_sb = {}
                for nm, dram in (("q", wq_d), ("k", wk_d), ("v", wv_d)):
                    wt = wpool.tile([128, KT, JW], F32R, name=f"w_{nm}")
                    nc.sync.dma_start(
                        out=wt, in_=dram.rearrange("(kt p) j -> p kt j", p=128).bitcast(F32R))
                    w_sb[nm] = wt

                x1r = x1t.rearrange("(kt p) n -> p kt n", p=128)
                x2r = x2t.rearrange("(kt p) n -> p kt n", p=128)

                def layernorm_into(psum, dst, bias_b, gb, bb_):
                    # per-head LN of a [128, 512] projection tile
                    if bias_b is not None:
                        src = ln_pool.tile([128, JW], F32, name="biased",
                                           tag="biased")
                        nc.vector.tensor_add(out=src, in0=psum, in1=bias_b)
                    else:
                        src = psum
                    stats = st_pool.tile([128, HG, 6], F32, name="stats")
                    for h in range(HG):
                        nc.vector.bn_stats(
                            out=stats[:, h, :],
                            in_=src[:, h * HDIM:(h + 1) * HDIM])
                    mv = st_pool.tile([128, HG, 2], F32, name="mv")
                    for h in range(HG):
                        nc.vector.bn_aggr(out=mv[:, h, :], in_=stats[:, h, :])
                    std = st_pool.tile([128, HG], F32, name="std")
                    nc.scalar.activation(
                        out=std, in_=mv[:, :, 1],
                        func=mybir.ActivationFunctionType.Sqrt,
                        bias=eps_sb, scale=1.0)
                    rstd = st_pool.tile([128, HG], F32, name="rstd")
                    nc.vector.reciprocal(out=rstd, in_=std)
                    for h in range(HG):
                        nc.vector.tensor_scalar(
                            out=dst[:, h * HDIM:(h + 1) * HDIM],
                            in0=src[:, h * HDIM:(h + 1) * HDIM],
                            scalar1=mv[:, h, 0:1], scalar2=rstd[:, h:h + 1],
                            op0=mybir.AluOpType.subtract,
                            op1=mybir.AluOpType.mult)
                    if gb is not None:
                        nc.vector.tensor_mul(out=dst, in0=dst, in1=gb)
                        nc.vector.tensor_add(out=dst, in0=dst, in1=bb_)

                for nt in range(nt_n):
                    nsl = slice(nt * 128, (nt + 1) * 128)
                    x1c = xc_pool.tile([128, KT, 128], F32R, name="x1c")
                    nc.sync.dma_start(out=x1c, in_=x1r[:, :, nsl].bitcast(F32R))
                    x2c = xc_pool.tile([128, KT, 128], F32R, name="x2c")
                    nc.sync.dma_start(out=x2c, in_=x2r[:, :, nsl].bitcast(F32R))

                    for nm, xc, dstT, bias_b, gb, bb_ in (
                        ("q", x1c, qT, bqb, gqb, btqb),
                        ("k", x2c, kT, bkb, gkb, btkb),
                    ):
                        ps = proj_ps.tile([128, JW], F32, name="ps", tag="ps")
                        for ct in range(KT):
                            nc.tensor.matmul(
                                ps, xc[:, ct, :], w_sb[nm][:, ct, :],
                                start=(ct == 0), stop=(ct == KT - 1))
                        ln = ln_pool.tile([128, JW], F32, name="ln", tag="ln")
                        layernorm_into(ps, ln, bias_b, gb, bb_)
                        for jt in range(4):
                            tp = tp_ps.tile([128, 128], F32, name="tp", tag="tp")
                            nc.tensor.transpose(
                                tp, ln[:, jt * 128:(jt + 1) * 128], eye_sb)
                            nc.any.tensor_copy(out=dstT[:, jt, nsl], in_=tp)

                    ps = proj_ps.tile([128, JW], F32, name="ps", tag="ps")
                    for ct in range(KT):
                        nc.tensor.matmul(
                            ps, x2c[:, ct, :], w_sb["v"][:, ct, :],
                            start=(ct == 0), stop=(ct == KT - 1))
                    psg = ps.rearrange("p (h d) -> p h d", h=HG)
                    if bvb is not None:
                        nc.vector.tensor_add(
                            out=vA[:, nt, :, 0:HDIM], in0=psg,
                            in1=bvb.rearrange("p (h d) -> p h d", h=HG))
                    else:
                        nc.vector.tensor_copy(out=vA[:, nt, :, 0:HDIM], in_=psg)

            # ---------------- attention ----------------
            with tc.tile_pool(name="sps", bufs=2, space="PSUM") as s_ps, \
                 tc.tile_pool(name="pvps", bufs=4, space="PSUM") as pv_ps, \
                 tc.tile_pool(name="psb", bufs=3) as p_pool, \
                 tc.tile_pool(name="nrm", bufs=3) as n_pool:
                for h in range(HG):
                    pt, bp = divmod(h, 2)
                    prows = slice(bp * 64, (bp + 1) * 64)
                    kTh = kT[prows, pt, :]
                    qTh = qT[prows, pt, :]
                    for blk in range(nblk):
                        pvs = [pv_ps.tile([65, 512], F32, name=f"pv{c2}",
                                          tag="pv") for c2 in range(nch)]
                        for m in range(nt_n):
                            s = s_ps.tile([128, sw], F32, name="s", tag="s")
                            for c2 in range(nch):
                                n0 = blk * sw + c2 * 512
                                nc.tensor.matmul(
                                    s[:, c2 * 512:(c2 + 1) * 512],
                                    kTh[:, m * 128:(m + 1) * 128],
                                    qTh[:, n0:n0 + 512],
                                    start=True, stop=True)
                            p = p_pool.tile([128, sw], F32R, name="p", tag="p")
                            nc.scalar.activation(
                                out=p, in_=s,
                                func=mybir.ActivationFunctionType.Exp,
                                scale=float(scale))
                            for c2 in range(nch):
                                nc.tensor.matmul(
                                    pvs[c2], vA[:, m, h, :],
                                    p[:, c2 * 512:(c2 + 1) * 512],
                                    start=(m == 0), stop=(m == nt_n - 1))
                        for c2 in range(nch):
                            den = n_pool.tile([1, 512], F32R, name="den",
                                              tag="den")
                            # f32r is bit-identical f32; only the PE multiply
                            # path rounds, so this is not a precision loss
                            with nc.allow_low_precision(reason="f32r==f32 bits"):
                                nc.vector.reciprocal(out=den,
                                                     in_=pvs[c2][64:65, :])
                            # broadcast recip across 64 partitions via a K=1
                            # matmul (ones column x recip row)
                            denb = pv_ps.tile([64, 512], F32, name="denb",
                                              tag="pv")
                            nc.tensor.matmul(
                                denb, ones64, den, start=True, stop=True)
                            denb_sb = n_pool.tile([64, 512], F32, name="denb_sb",
                                                  tag="denb_sb")
                            nc.any.tensor_copy(out=denb_sb, in_=denb)
                            osb = n_pool.tile([64, 512], F32, name="osb",
                                              tag="osb")
                            nc.vector.tensor_mul(
                                out=osb, in0=pvs[c2][0:64, :], in1=denb_sb)
                            n0 = blk * sw + c2 * 512
                            nc.sync.dma_start(
                                out=out_d[h * HDIM:(h + 1) * HDIM,
                                          n0:n0 + 512],
                                in_=osb)
    split_multi_waits(nc)
    return nc


def shard_inputs(x1, x2, wq, bq, wkv, bkv, gamma_q, beta_q, gamma_k, beta_k,
                 flags, n_seq=NSEQ):
    has_bq, has_bkv, has_gbq, has_gbk = flags
    eye = np.eye(128, dtype=np.float32)
    in_maps = []
    for core in range(NCORES):
        b, g = divmod(core, 2)
        jsl = slice(g * JW, (g + 1) * JW)
        m = {
            "x1t": np.ascontiguousarray(x1[b, :n_seq].T),
            "x2t": np.ascontiguousarray(x2[b, :n_seq].T),
            "wq": np.ascontiguousarray(wq[:, jsl]),
            "wk": np.ascontiguousarray(wkv[:, jsl]),
            "wv": np.ascontiguousarray(wkv[:, DIM + g * JW:DIM + (g + 1) * JW]),
            "eye": eye,
            "onesv": np.ones(128, dtype=np.float32),
        }
        if has_bq:
            m["bq"] = np.ascontiguousarray(bq[jsl])
        if has_bkv:
            m["bk"] = np.ascontiguousarray(bkv[jsl])
            m["bv"] = np.ascontiguousarray(bkv[DIM + g * JW:DIM + (g + 1) * JW])
        if has_gbq:
            m["gq"] = np.tile(gamma_q, HG).astype(np.float32)
            m["betq"] = np.tile(beta_q, HG).astype(np.float32)
        if has_gbk:
            m["gk"] = np.tile(gamma_k, HG).astype(np.float32)
            m["betk"] = np.tile(beta_k, HG).astype(np.float32)
        in_maps.append(m)
    return in_maps


def kernel(x1, x2, wq, bq, wkv, bkv, gamma_q, beta_q, gamma_k, beta_k):
    x1 = np.asarray(x1, dtype=np.float32)
    x2 = np.asarray(x2, dtype=np.float32)
    wq = np.asarray(wq, dtype=np.float32)
    bq = np.asarray(bq, dtype=np.float32)
    wkv = np.asarray(wkv, dtype=np.float32)
    bkv = np.asarray(bkv, dtype=np.float32)
    gamma_q = np.asarray(gamma_q, dtype=np.float32)
    beta_q = np.asarray(beta_q, dtype=np.float32)
    gamma_k = np.asarray(gamma_k, dtype=np.float32)
    beta_k = np.asarray(beta_k, dtype=np.float32)

    flags = (
        bool(np.any(bq)),
        bool(np.any(bkv)),
        not (np.all(gamma_q == 1.0) and np.all(beta_q == 0.0)),
        not (np.all(gamma_k == 1.0) and np.all(beta_k == 0.0)),
    )
    nc = build(NSEQ, *flags)
    in_maps = shard_inputs(x1, x2, wq, bq, wkv, bkv, gamma_q, beta_q,
                           gamma_k, beta_k, flags)
    trace = bool(int(os.environ.get("KERNEL_TRACE", "0")))
    res = run_bass_kernel_spmd(nc, in_maps, core_ids=list(range(NCORES)),
                               trace=trace)
    global LAST_RESULTS
    LAST_RESULTS = res
    out = np.empty((B, NSEQ, DIM), dtype=np.float32)
    for core in range(NCORES):
        b, g = divmod(core, 2)
        out[b, :, g * JW:(g + 1) * JW] = res.results[core]["outT"].T
    return out
